# revision 13
# baseline (speedup 1.0000x reference)
"""Trainium2 Bass kernel for nn_AGRNNCell (attention + top-k + GCN-gated GRU cell).

Sharding: batch dim B=8 across 8 NeuronCores (one graph per core); the small
weight matrices are replicated.

Per-core algorithm (N=2000 nodes, H=64, D=128, K=20). Most tensors are kept in
transposed "feature-major" layout [feat, node]:

  xsT   = [lin_W^T @ xT + lin_b ; stateT]                      [128, N]
  qsT   = (Wq^T @ xsT + bq) / 8,  kT = Wk^T @ xsT + bk         [64, N]
  S_ci  = qsT[:,chunk]^T @ kT  (f32 PSUM, 16 row-chunks)       [128, N]
  top-k: 3x max8 + 2x match_replace -> rank-20 value t_p per row; Z = sum of
         exp over the kept 20 values (shift-free exp; scores are O(1)).
  A_row = exp(S) * (S >= t) / Z   (attn, bf16)
  AT    = xbar-DMA transpose of A_row (bf16) = attn^T; also the A output
          (cast bf16->f32 during the output DMA).
  Dense-GCN trick: the masked softmax has exactly 20 nonzeros per row summing
  to 1, so GCNConv's degree is exactly 2 (+O(1e-7)) and
  prop(h) = dinv*(Attn+I)(dinv*h) = 0.5*(attn @ h + h).
  ctx^T = v^T-stationary matmuls streaming AT
  xx^T  = Wo^T ctx^T + bo + xsT; LayerNorm over features via PE column-sums
  zr^T  = sigmoid(m12 + h12 + [g1_b;g2_b]),  h12 = 0.5 * xxn @ [g1_W|g2_W],
          m12 = attn @ h12  (streams AT with h12 in natural layout)
  hc^T  = tanh(m3 + h3 + up_b),  h3 = 0.5 * (xxn @ up1 + (z*state) @ up2)
  h     = hc + r*(state - hc)

kernel(**inputs) takes FULL unsharded inputs, returns (h, A) like reference.
"""

import numpy as np

N = 2000
DIN = 32
H = 64
D = 128
K = 20
NCH = 16  # row chunks of 128 (last is 80)
CH = [(ci * 128, min(128, N - ci * 128)) for ci in range(NCH)]
NPAD = 2048  # A_row free-dim pad so every transpose block is 128 wide

# packed weights layout: name -> (partitions, col offset, col width)
WLAYOUT = {
    "lin_W": (32, 0, 64), "w_q": (128, 64, 64), "w_k": (128, 128, 64),
    "w_v": (128, 192, 64), "w_o": (64, 256, 128), "g12": (128, 384, 128),
    "up1": (128, 512, 64), "up2": (64, 576, 64), "ident": (128, 640, 128),
    "linb": (64, 768, 1), "bq8": (64, 769, 1), "bk": (64, 770, 1),
    "bv": (64, 771, 1), "bo": (128, 772, 1), "gb12": (128, 773, 1),
    "upb": (64, 774, 1), "lng": (128, 775, 1), "lnb": (128, 776, 1),
    "ones128": (128, 777, 1),
}
WPACK = 784

_PROG = None


def _chunks512(width):
    out, o = [], 0
    while o < width:
        w = min(512, width - o)
        out.append((o, w))
        o += w
    return out


def _build_program():
    from contextlib import ExitStack

    import concourse.bacc as bacc
    import concourse.mybir as mybir
    import concourse.tile as tile

    f32 = mybir.dt.float32
    bf16 = mybir.dt.bfloat16
    AF = mybir.ActivationFunctionType
    OP = mybir.AluOpType

    nc = bacc.Bacc("TRN2", target_bir_lowering=False, debug=False, num_devices=8)

    din = {}
    for name, shape in [
        ("xT", [DIN, N]), ("stT", [H, N]), ("wpack", [128, WPACK]),
    ]:
        din[name] = nc.dram_tensor(name, shape, f32, kind="ExternalInput").ap()
    h_out = nc.dram_tensor("h_out", [N, H], f32, kind="ExternalOutput").ap()
    A_out = nc.dram_tensor("A_out", [N, N], f32, kind="ExternalOutput").ap()

    with tile.TileContext(nc) as tc, ExitStack() as top:
        # ---------------- persistent SBUF ----------------
        pers = top.enter_context(tc.tile_pool(name="pers", bufs=1))
        wk = pers.tile([128, WPACK], f32, tag="wpack")
        nc.sync.dma_start(wk[:], din["wpack"])
        sb = {name: wk[0:p, c0:c0 + w] for name, (p, c0, w) in WLAYOUT.items()}

        xs0 = pers.tile([128, N], f32, tag="xs0")  # [0:32]=xT, [64:128]=stateT
        nc.sync.dma_start(xs0[0:DIN, :], din["xT"])
        nc.sync.dma_start(xs0[64:128, :], din["stT"])
        stT0 = pers.tile([H, N], f32, tag="stT0")  # base-0 copy for DVE ops
        nc.sync.dma_start(stT0[0:H, :], din["stT"])

        xsT = pers.tile([128, N], f32, tag="xsT")
        qsT = pers.tile([H, N], f32, tag="qsT")
        kT = pers.tile([H, N], f32, tag="kT")
        vnat = pers.tile([128, NCH * H], bf16, tag="vnat")
        AT = pers.tile([128, NCH * N], bf16, tag="AT")  # strip fj = [:, fj*N:(fj+1)*N]
        xxn = pers.tile([128, N], f32, tag="xxn")

        # ---------------- P1: projections ----------------
        with ExitStack() as p1:
            ps_a = p1.enter_context(tc.tile_pool(name="ps_a", bufs=1, space="PSUM"))
            ps_tr = p1.enter_context(tc.tile_pool(name="ps_tr", bufs=2, space="PSUM"))
            tmp1 = p1.enter_context(tc.tile_pool(name="tmp1", bufs=1))

            ps = ps_a.tile([H, NPAD], f32, tag="ps1")
            for o, w in _chunks512(N):
                nc.tensor.matmul(ps[0:H, o:o + w], sb["lin_W"][:],
                                 xs0[0:DIN, o:o + w], start=True, stop=True)
            nc.scalar.activation(xsT[0:H, :], ps[0:H, 0:N], AF.Identity,
                                 bias=sb["linb"][:])
            nc.vector.tensor_copy(xsT[64:128, :], xs0[64:128, :])

            for wname, bname, dst, scale in [("w_q", "bq8", qsT, 0.125),
                                             ("w_k", "bk", kT, 1.0)]:
                ps = ps_a.tile([H, NPAD], f32, tag="ps1")
                for o, w in _chunks512(N):
                    nc.tensor.matmul(ps[0:H, o:o + w], sb[wname][:],
                                     xsT[:, o:o + w], start=True, stop=True)
                nc.scalar.activation(dst[0:H, :], ps[0:H, 0:N], AF.Identity,
                                     bias=sb[bname][:], scale=scale)

            vT = tmp1.tile([H, N], f32, tag="vT")
            ps = ps_a.tile([H, NPAD], f32, tag="ps1")
            for o, w in _chunks512(N):
                nc.tensor.matmul(ps[0:H, o:o + w], sb["w_v"][:], xsT[:, o:o + w],
                                 start=True, stop=True)
            nc.scalar.activation(vT[0:H, :], ps[0:H, 0:N], AF.Identity, bias=sb["bv"][:])

            for fj, (r0, rc) in enumerate(CH):
                pst = ps_tr.tile([128, H], f32, tag="pst")
                nc.tensor.transpose(pst[0:rc, 0:H], vT[0:H, r0:r0 + rc],
                                    sb["ident"][0:H, 0:H])
                nc.scalar.activation(vnat[0:rc, fj * H:(fj + 1) * H],
                                     pst[0:rc, 0:H], AF.Copy)

        # -------- P2: scores, top-k, attention, transpose --------
        with ExitStack() as p2:
            ps_s = p2.enter_context(tc.tile_pool(name="ps_s", bufs=2, space="PSUM"))
            scw = p2.enter_context(tc.tile_pool(name="scw", bufs=2))
            vps = p2.enter_context(tc.tile_pool(name="vps", bufs=2))
            epool = p2.enter_context(tc.tile_pool(name="epool", bufs=2))
            apool = p2.enter_context(tc.tile_pool(name="apool", bufs=3))

            for ci, (r0, rc) in enumerate(CH):
                ps = ps_s.tile([128, NPAD], f32, tag="S")
                for o, w in _chunks512(N):
                    nc.tensor.matmul(ps[0:rc, o:o + w], qsT[0:H, r0:r0 + rc],
                                     kT[0:H, o:o + w], start=True, stop=True)

                vv = vps.tile([128, 24], f32, tag="vv")
                scp = scw.tile([128, N], f32, tag="scp")
                nc.vector.max(out=vv[0:rc, 0:8], in_=ps[0:rc, 0:N])
                nc.vector.match_replace(out=scp[0:rc, :], in_to_replace=vv[0:rc, 0:8],
                                        in_values=ps[0:rc, 0:N], imm_value=-1e30)
                nc.vector.max(out=vv[0:rc, 8:16], in_=scp[0:rc, :])
                nc.vector.match_replace(out=scp[0:rc, :], in_to_replace=vv[0:rc, 8:16],
                                        in_values=scp[0:rc, :], imm_value=-1e30)
                nc.vector.max(out=vv[0:rc, 16:24], in_=scp[0:rc, :])

                # E = exp(S) (independent of the top-k path, overlaps max8);
                # fused A_unnorm = (S >= t) * E with accum Z = rowsum;
                # then A_row = A_unnorm / Z (bf16 4x pass).
                et = epool.tile([128, N], f32, tag="E")
                nc.scalar.activation(et[0:rc, :], ps[0:rc, 0:N], AF.Exp)
                zs = vps.tile([128, 1], f32, tag="zs")
                rz = vps.tile([128, 1], f32, tag="rz")
                ar = apool.tile([128, NPAD], bf16, tag="ar")
                nc.vector.scalar_tensor_tensor(out=ar[0:rc, 0:N], in0=ps[0:rc, 0:N],
                                               scalar=vv[0:rc, 19:20],
                                               in1=et[0:rc, :],
                                               op0=OP.is_ge, op1=OP.mult,
                                               accum_out=zs[0:rc, :])
                nc.vector.reciprocal(rz[0:rc, :], zs[0:rc, :])
                nc.vector.tensor_scalar_mul(ar[0:rc, 0:N], ar[0:rc, 0:N],
                                            rz[0:rc, 0:1])

                at3d = AT[:].rearrange("p (f n) -> p f n", f=NCH)
                nc.sync.dma_start_transpose(out=at3d[:, :, r0:r0 + rc],
                                            in_=ar[0:rc, 0:NPAD])

        # A output: bf16 strips -> f32 rows (cast in SWDGE DMA)
        for fj, (r0, rc) in enumerate(CH):
            nc.gpsimd.dma_start(A_out[r0:r0 + rc, :], AT[0:rc, fj * N: fj * N + N])

        # ---------------- P3a: ctx, xx, LayerNorm ----------------
        with ExitStack() as p3:
            tmpb = p3.enter_context(tc.tile_pool(name="tmpb", bufs=1))
            tmpa = p3.enter_context(tc.tile_pool(name="tmpa", bufs=1))

            with ExitStack() as s1:
                ps_c = s1.enter_context(tc.tile_pool(name="ps_c", bufs=1, space="PSUM"))
                ctxT = tmpb.tile([H, N], f32, tag="ctxT")
                ps = ps_c.tile([H, NPAD], f32, tag="psc")
                for o, w in _chunks512(N):
                    for fj, (jr0, jrc) in enumerate(CH):
                        nc.tensor.matmul(ps[0:H, o:o + w],
                                         vnat[0:jrc, fj * H:(fj + 1) * H],
                                         AT[0:jrc, fj * N + o: fj * N + o + w],
                                         start=(fj == 0), stop=(fj == NCH - 1))
                nc.scalar.activation(ctxT[0:H, :], ps[0:H, 0:N], AF.Copy)

                psx = ps_c.tile([128, NPAD], f32, tag="psx")
                for o, w in _chunks512(N):
                    nc.tensor.matmul(psx[0:D, o:o + w], sb["w_o"][:],
                                     ctxT[0:H, o:o + w], start=True, stop=True)
                xxt = tmpb.tile([128, N], f32, tag="xxt")
                nc.scalar.activation(xxt[:, :], psx[0:D, 0:N], AF.Identity,
                                     bias=sb["bo"][:])
                nc.vector.tensor_add(xxt[:, :], xxt[:, :], xsT[:, :])
                sqt = tmpa.tile([128, N], f32, tag="sqt")
                nc.scalar.activation(sqt[:, :], xxt[:, :], AF.Square)

            with ExitStack() as s2:
                ps_l = s2.enter_context(tc.tile_pool(name="ps_l", bufs=1, space="PSUM"))
                psl = ps_l.tile([1, 2 * NPAD], f32, tag="psl")
                for o, w in _chunks512(N):
                    nc.tensor.matmul(psl[0:1, o:o + w], sb["ones128"][:],
                                     xxt[:, o:o + w], start=True, stop=True)
                    nc.tensor.matmul(psl[0:1, NPAD + o: NPAD + o + w],
                                     sb["ones128"][:], sqt[:, o:o + w],
                                     start=True, stop=True)
                # mu = colsum/128, msq = colsum(x^2)/128, isd = 1/sqrt(var+eps)
                lnv = tmpa.tile([1, 2 * NPAD], f32, tag="lnv")
                nc.vector.tensor_scalar(out=lnv[0:1, :], in0=psl[0:1, :],
                                        scalar1=1.0 / 128.0, scalar2=None,
                                        op0=OP.mult)
                lntmp = tmpa.tile([1, N], f32, tag="lntmp")
                nc.vector.tensor_mul(lntmp[0:1, :], lnv[0:1, 0:N], lnv[0:1, 0:N])
                nc.vector.tensor_sub(lnv[0:1, NPAD:NPAD + N],
                                     lnv[0:1, NPAD:NPAD + N], lntmp[0:1, :])
                nc.vector.tensor_scalar_add(lnv[0:1, NPAD:NPAD + N],
                                            lnv[0:1, NPAD:NPAD + N], 1e-6)
                nc.scalar.activation(lntmp[0:1, :], lnv[0:1, NPAD:NPAD + N],
                                     AF.Sqrt)
                nc.vector.reciprocal(lnv[0:1, NPAD:NPAD + N], lntmp[0:1, :])

                bc = tmpa.tile([128, 2 * NPAD], f32, tag="bc")
                nc.gpsimd.partition_broadcast(bc[0:128, 0:N], lnv[0:1, 0:N])
                nc.gpsimd.partition_broadcast(bc[0:128, NPAD:NPAD + N],
                                              lnv[0:1, NPAD:NPAD + N])
                nc.vector.tensor_sub(xxn[:, :], xxt[:, :], bc[:, 0:N])
                nc.vector.tensor_mul(xxn[:, :], xxn[:, :], bc[:, NPAD:NPAD + N])
                nc.vector.tensor_scalar(out=xxn[:, :], in0=xxn[:, :],
                                        scalar1=sb["lng"][:], scalar2=sb["lnb"][:],
                                        op0=OP.mult, op1=OP.add)

        # ---------------- P3b: gates and update ----------------
        with ExitStack() as p4:
            wrk = p4.enter_context(tc.tile_pool(name="wrk", bufs=1))

            # h12 = 0.5 * xxn @ [g1_W | g2_W]  (T-layout + natural bf16)
            h12T = wrk.tile([128, N], f32, tag="h12T")
            h12n = wrk.tile([128, NCH * D], bf16, tag="h12n")
            with ExitStack() as s3:
                ps_h = s3.enter_context(tc.tile_pool(name="ps_h", bufs=1, space="PSUM"))
                ps_t2 = s3.enter_context(tc.tile_pool(name="ps_t2", bufs=2, space="PSUM"))
                ps = ps_h.tile([128, NPAD], f32, tag="psh")
                for o, w in _chunks512(N):
                    nc.tensor.matmul(ps[0:D, o:o + w], sb["g12"][:],
                                     xxn[:, o:o + w], start=True, stop=True)
                nc.scalar.activation(h12T[:, :], ps[0:D, 0:N], AF.Identity, scale=0.5)
                for fj, (r0, rc) in enumerate(CH):
                    pst = ps_t2.tile([128, D], f32, tag="pst2")
                    nc.tensor.transpose(pst[0:rc, 0:D], h12T[0:D, r0:r0 + rc],
                                        sb["ident"][:])
                    nc.scalar.activation(h12n[0:rc, fj * D:(fj + 1) * D],
                                         pst[0:rc, 0:D], AF.Copy)

            # zr = sigmoid(attn @ h12 + h12 + gb12); z/r as separate base-0 tiles
            zT = wrk.tile([H, N], f32, tag="zT")
            rT = wrk.tile([H, N], f32, tag="rT")
            scr = wrk.tile([128, N], f32, tag="scr")
            with ExitStack() as s4:
                ps_m = s4.enter_context(tc.tile_pool(name="ps_m", bufs=1, space="PSUM"))
                ps = ps_m.tile([128, NPAD], f32, tag="psm")
                for o, w in _chunks512(N):
                    for fj, (jr0, jrc) in enumerate(CH):
                        nc.tensor.matmul(ps[0:D, o:o + w],
                                         h12n[0:jrc, fj * D:(fj + 1) * D],
                                         AT[0:jrc, fj * N + o: fj * N + o + w],
                                         start=(fj == 0), stop=(fj == NCH - 1))
                nc.vector.tensor_add(scr[:, :], ps[0:D, 0:N], h12T[:, :])
                nc.scalar.activation(zT[0:H, :], scr[0:H, :], AF.Sigmoid,
                                     bias=sb["gb12"][0:H, :])
                nc.scalar.activation(rT[0:H, :], scr[64:128, :], AF.Sigmoid,
                                     bias=sb["gb12"][64:128, :])

            # h3 = 0.5 * (xxn @ up1 + (z*state) @ up2)
            zst = wrk.tile([H, N], f32, tag="zst")
            nc.vector.tensor_mul(zst[0:H, :], zT[0:H, :], stT0[0:H, :])
            h3T = wrk.tile([H, N], f32, tag="h3T")
            h3n = wrk.tile([128, NCH * H], bf16, tag="h3n")
            with ExitStack() as s5:
                ps_3 = s5.enter_context(tc.tile_pool(name="ps_3", bufs=1, space="PSUM"))
                ps_t3 = s5.enter_context(tc.tile_pool(name="ps_t3", bufs=2, space="PSUM"))
                ps = ps_3.tile([H, NPAD], f32, tag="ps3")
                for o, w in _chunks512(N):
                    nc.tensor.matmul(ps[0:H, o:o + w], sb["up1"][:],
                                     xxn[:, o:o + w], start=True, stop=False)
                    nc.tensor.matmul(ps[0:H, o:o + w], sb["up2"][:],
                                     zst[0:H, o:o + w], start=False, stop=True)
                nc.scalar.activation(h3T[0:H, :], ps[0:H, 0:N], AF.Identity, scale=0.5)
                for fj, (r0, rc) in enumerate(CH):
                    pst = ps_t3.tile([128, H], f32, tag="pst3")
                    nc.tensor.transpose(pst[0:rc, 0:H], h3T[0:H, r0:r0 + rc],
                                        sb["ident"][0:H, 0:H])
                    nc.scalar.activation(h3n[0:rc, fj * H:(fj + 1) * H],
                                         pst[0:rc, 0:H], AF.Copy)

            # hc = tanh(attn @ h3 + h3 + up_b);  h = hc + r*(state - hc)
            hcT = wrk.tile([H, N], f32, tag="hcT")
            hT = wrk.tile([H, N], f32, tag="hT")
            with ExitStack() as s6:
                ps_m3 = s6.enter_context(tc.tile_pool(name="ps_m3", bufs=1, space="PSUM"))
                ps = ps_m3.tile([H, NPAD], f32, tag="psm3")
                for o, w in _chunks512(N):
                    for fj, (jr0, jrc) in enumerate(CH):
                        nc.tensor.matmul(ps[0:H, o:o + w],
                                         h3n[0:jrc, fj * H:(fj + 1) * H],
                                         AT[0:jrc, fj * N + o: fj * N + o + w],
                                         start=(fj == 0), stop=(fj == NCH - 1))
                nc.vector.tensor_add(scr[0:H, :], ps[0:H, 0:N], h3T[0:H, :])
                nc.scalar.activation(hcT[0:H, :], scr[0:H, :], AF.Tanh,
                                     bias=sb["upb"][:])
            nc.vector.tensor_sub(hT[0:H, :], stT0[0:H, :], hcT[0:H, :])
            nc.vector.tensor_mul(hT[0:H, :], hT[0:H, :], rT[0:H, :])
            nc.vector.tensor_add(hT[0:H, :], hT[0:H, :], hcT[0:H, :])

            # h output: transpose to natural layout and DMA out
            with ExitStack() as s7:
                ps_t4 = s7.enter_context(tc.tile_pool(name="ps_t4", bufs=2, space="PSUM"))
                hn = p4.enter_context(tc.tile_pool(name="hn", bufs=2))
                for fj, (r0, rc) in enumerate(CH):
                    pst = ps_t4.tile([128, H], f32, tag="pst4")
                    nc.tensor.transpose(pst[0:rc, 0:H], hT[0:H, r0:r0 + rc],
                                        sb["ident"][0:H, 0:H])
                    hnt = hn.tile([128, H], f32, tag="hnt")
                    nc.scalar.activation(hnt[0:rc, :], pst[0:rc, 0:H], AF.Copy)
                    nc.sync.dma_start(h_out[r0:r0 + rc, :], hnt[0:rc, :])

    nc.compile()
    return nc


def _prep_shards(inputs):
    g = lambda k: np.ascontiguousarray(np.asarray(inputs[k]), dtype=np.float32)
    x, state = g("x"), g("state")
    B = x.shape[0]
    assert x.shape == (B, N, DIN) and state.shape == (B, N, H)
    assert int(np.asarray(inputs["top_k"])) == K

    col = lambda a: np.asarray(a, np.float32).reshape(-1, 1)
    mats = {
        "lin_W": g("lin_W"), "w_q": g("Wq"), "w_k": g("Wk"), "w_v": g("Wv"),
        "w_o": g("Wo"),
        "g12": np.concatenate([g("g1_W"), g("g2_W")], axis=1),
        "up1": g("up_W")[:D], "up2": g("up_W")[D:],
        "linb": col(inputs["lin_b"]), "bq8": col(np.asarray(inputs["bq"]) / 8.0),
        "bk": col(inputs["bk"]), "bv": col(inputs["bv"]), "bo": col(inputs["bo"]),
        "gb12": col(np.concatenate([np.asarray(inputs["g1_b"]),
                                    np.asarray(inputs["g2_b"])])),
        "upb": col(inputs["up_b"]),
        "lng": col(inputs["ln_g"]), "lnb": col(inputs["ln_b"]),
        "ident": np.eye(128, dtype=np.float32),
        "ones128": np.ones((128, 1), dtype=np.float32),
    }
    wpack = np.zeros((128, WPACK), dtype=np.float32)
    for name, (p, c0, w) in WLAYOUT.items():
        a = mats[name]
        assert a.shape == (p, w), (name, a.shape, (p, w))
        wpack[0:p, c0:c0 + w] = a
    common = {"wpack": wpack}
    in_maps = []
    for c in range(B):
        m = dict(common)
        m["xT"] = np.ascontiguousarray(x[c].T)
        m["stT"] = np.ascontiguousarray(state[c].T)
        in_maps.append(m)
    return in_maps, B


def kernel(**inputs):
    global _PROG
    from concourse.bass_utils import run_bass_kernel_spmd

    if _PROG is None:
        _PROG = _build_program()
    nc = _PROG

    in_maps, B = _prep_shards(inputs)
    assert B == 8, f"expected B=8, got {B}"
    res = run_bass_kernel_spmd(nc, in_maps, core_ids=list(range(B)))
    h = np.stack([res.results[c]["h_out"] for c in range(B)])
    A = np.stack([res.results[c]["A_out"] for c in range(B)])
    return h, A


# revision 15
# speedup vs baseline: 1.1994x; 1.1994x over previous
"""Trainium2 Bass kernel for nn_AGRNNCell (attention + top-k + GCN-gated GRU cell).

Sharding: batch dim B=8 across 8 NeuronCores (one graph per core); the small
weight matrices are replicated.

Per-core algorithm (N=2000 nodes, H=64, D=128, K=20). Most tensors are kept in
transposed "feature-major" layout [feat, node]:

  xsT   = [lin_W^T @ xT + lin_b ; stateT]                      [128, N]
  qsT   = (Wq^T @ xsT + bq) / 8,  kT = Wk^T @ xsT + bk         [64, N]
  S_ci  = qsT[:,chunk]^T @ kT  (f32 PSUM, 16 row-chunks)       [128, N]
  top-k: 3x max8 + 2x match_replace -> rank-20 value t_p per row; Z = sum of
         exp over the kept 20 values (shift-free exp; scores are O(1)).
  A_row = exp(S) * (S >= t) / Z   (attn, bf16)
  AT    = xbar-DMA transpose of A_row (bf16) = attn^T; also the A output
          (cast bf16->f32 during the output DMA).
  Dense-GCN trick: the masked softmax has exactly 20 nonzeros per row summing
  to 1, so GCNConv's degree is exactly 2 (+O(1e-7)) and
  prop(h) = dinv*(Attn+I)(dinv*h) = 0.5*(attn @ h + h).
  ctx^T = v^T-stationary matmuls streaming AT
  xx^T  = Wo^T ctx^T + bo + xsT; LayerNorm over features via PE column-sums
  zr^T  = sigmoid(m12 + h12 + [g1_b;g2_b]),  h12 = 0.5 * xxn @ [g1_W|g2_W],
          m12 = attn @ h12  (streams AT with h12 in natural layout)
  hc^T  = tanh(m3 + h3 + up_b),  h3 = 0.5 * (xxn @ up1 + (z*state) @ up2)
  h     = hc + r*(state - hc)

kernel(**inputs) takes FULL unsharded inputs, returns (h, A) like reference.
"""

import numpy as np

N = 2000
DIN = 32
H = 64
D = 128
K = 20
NCH = 16  # row chunks of 128 (last is 80)
CH = [(ci * 128, min(128, N - ci * 128)) for ci in range(NCH)]
NPAD = 2048  # A_row free-dim pad so every transpose block is 128 wide

# packed weights layout: name -> (partitions, col offset, col width)
WLAYOUT = {
    "lin_W": (32, 0, 64), "w_q": (128, 64, 64), "w_k": (128, 128, 64),
    "w_v": (128, 192, 64), "w_o": (64, 256, 128), "g12": (128, 384, 128),
    "up1": (128, 512, 64), "up2": (64, 576, 64), "ident": (128, 640, 128),
    "linb": (64, 768, 1), "bq8": (64, 769, 1), "bk": (64, 770, 1),
    "bv": (64, 771, 1), "bo": (128, 772, 1), "gb12": (128, 773, 1),
    "upb": (64, 774, 1), "lng": (128, 775, 1), "lnb": (128, 776, 1),
    "ones128": (128, 777, 1),
}
WPACK = 784

_PROG = None


def _chunks512(width):
    out, o = [], 0
    while o < width:
        w = min(512, width - o)
        out.append((o, w))
        o += w
    return out


def _build_program():
    from contextlib import ExitStack

    import concourse.bacc as bacc
    import concourse.mybir as mybir
    import concourse.tile as tile

    f32 = mybir.dt.float32
    bf16 = mybir.dt.bfloat16
    AF = mybir.ActivationFunctionType
    OP = mybir.AluOpType

    nc = bacc.Bacc("TRN2", target_bir_lowering=False, debug=False, num_devices=8)

    din = {}
    for name, shape in [
        ("xT", [DIN, N]), ("stT", [H, N]), ("wpack", [128, WPACK]),
    ]:
        din[name] = nc.dram_tensor(name, shape, f32, kind="ExternalInput").ap()
    h_out = nc.dram_tensor("h_out", [N, H], f32, kind="ExternalOutput").ap()
    A_out = nc.dram_tensor("A_out", [N, N], f32, kind="ExternalOutput").ap()

    with tile.TileContext(nc) as tc, ExitStack() as top:
        # ---------------- persistent SBUF ----------------
        pers = top.enter_context(tc.tile_pool(name="pers", bufs=1))
        wk = pers.tile([128, WPACK], f32, tag="wpack")
        nc.sync.dma_start(wk[:], din["wpack"])
        sb = {name: wk[0:p, c0:c0 + w] for name, (p, c0, w) in WLAYOUT.items()}

        xs0 = pers.tile([128, N], f32, tag="xs0")  # [0:32]=xT, [64:128]=stateT
        nc.sync.dma_start(xs0[0:DIN, :], din["xT"])
        nc.sync.dma_start(xs0[64:128, :], din["stT"])
        stT0 = pers.tile([H, N], f32, tag="stT0")  # base-0 copy for DVE ops
        nc.sync.dma_start(stT0[0:H, :], din["stT"])

        xsT = pers.tile([128, N], f32, tag="xsT")
        qsT = pers.tile([H, N], f32, tag="qsT")
        kT = pers.tile([H, N], f32, tag="kT")
        vnat = pers.tile([128, NCH * H], bf16, tag="vnat")
        AT = pers.tile([128, NCH * N], bf16, tag="AT")  # strip fj = [:, fj*N:(fj+1)*N]
        xxn = pers.tile([128, N], f32, tag="xxn")

        # ---------------- P1: projections ----------------
        with ExitStack() as p1:
            ps_a = p1.enter_context(tc.tile_pool(name="ps_a", bufs=4, space="PSUM"))
            ps_tr = p1.enter_context(tc.tile_pool(name="ps_tr", bufs=2, space="PSUM"))
            tmp1 = p1.enter_context(tc.tile_pool(name="tmp1", bufs=1))

            for o, w in _chunks512(N):
                psq = ps_a.tile([H, 512], f32, tag="ps1")
                nc.tensor.matmul(psq[0:H, 0:w], sb["lin_W"][:],
                                 xs0[0:DIN, o:o + w], start=True, stop=True)
                nc.scalar.activation(xsT[0:H, o:o + w], psq[0:H, 0:w], AF.Identity,
                                     bias=sb["linb"][:])
            nc.vector.tensor_copy(xsT[64:128, :], xs0[64:128, :])

            vT = tmp1.tile([H, N], f32, tag="vT")
            for wname, bname, dst, scale in [("w_q", "bq8", qsT, 0.125),
                                             ("w_k", "bk", kT, 1.0),
                                             ("w_v", "bv", vT, 1.0)]:
                for o, w in _chunks512(N):
                    psq = ps_a.tile([H, 512], f32, tag="ps1")
                    nc.tensor.matmul(psq[0:H, 0:w], sb[wname][:],
                                     xsT[:, o:o + w], start=True, stop=True)
                    nc.scalar.activation(dst[0:H, o:o + w], psq[0:H, 0:w],
                                         AF.Identity, bias=sb[bname][:], scale=scale)

            for fj, (r0, rc) in enumerate(CH):
                pst = ps_tr.tile([128, H], f32, tag="pst")
                nc.tensor.transpose(pst[0:rc, 0:H], vT[0:H, r0:r0 + rc],
                                    sb["ident"][0:H, 0:H])
                nc.scalar.activation(vnat[0:rc, fj * H:(fj + 1) * H],
                                     pst[0:rc, 0:H], AF.Copy)

        # -------- P2: scores, top-k, attention, transpose --------
        with ExitStack() as p2:
            ps_s = p2.enter_context(tc.tile_pool(name="ps_s", bufs=2, space="PSUM"))
            scw = p2.enter_context(tc.tile_pool(name="scw", bufs=2))
            vps = p2.enter_context(tc.tile_pool(name="vps", bufs=2))
            epool = p2.enter_context(tc.tile_pool(name="epool", bufs=2))
            apool = p2.enter_context(tc.tile_pool(name="apool", bufs=3))

            for ci, (r0, rc) in enumerate(CH):
                ps = ps_s.tile([128, NPAD], f32, tag="S")
                for o, w in _chunks512(N):
                    nc.tensor.matmul(ps[0:rc, o:o + w], qsT[0:H, r0:r0 + rc],
                                     kT[0:H, o:o + w], start=True, stop=True)

                vv = vps.tile([128, 24], f32, tag="vv")
                scp = scw.tile([128, N], f32, tag="scp")
                nc.vector.max(out=vv[0:rc, 0:8], in_=ps[0:rc, 0:N])
                nc.vector.match_replace(out=scp[0:rc, :], in_to_replace=vv[0:rc, 0:8],
                                        in_values=ps[0:rc, 0:N], imm_value=-1e30)
                nc.vector.max(out=vv[0:rc, 8:16], in_=scp[0:rc, :])
                nc.vector.match_replace(out=scp[0:rc, :], in_to_replace=vv[0:rc, 8:16],
                                        in_values=scp[0:rc, :], imm_value=-1e30)
                nc.vector.max(out=vv[0:rc, 16:24], in_=scp[0:rc, :])

                # E = exp(S) (ACT, overlaps max8 rounds; last PSUM reader).
                # Mask on E itself: S >= t  <=>  E >= exp(t) (exp monotone;
                # LUT collisions at the boundary are ~1e-7-probability).
                # Fused A_unnorm = (E >= exp(t)) * E with accum Z = rowsum,
                # then A_row = A_unnorm / Z (bf16 4x pass).
                et = epool.tile([128, N], f32, tag="E")
                nc.scalar.activation(et[0:rc, :], ps[0:rc, 0:N], AF.Exp)
                et20 = vps.tile([128, 1], f32, tag="et20")
                nc.scalar.activation(et20[0:rc, :], vv[0:rc, 19:20], AF.Exp)
                zs = vps.tile([128, 1], f32, tag="zs")
                rz = vps.tile([128, 1], f32, tag="rz")
                ar = apool.tile([128, NPAD], bf16, tag="ar")
                nc.vector.scalar_tensor_tensor(out=ar[0:rc, 0:N], in0=et[0:rc, :],
                                               scalar=et20[0:rc, 0:1],
                                               in1=et[0:rc, :],
                                               op0=OP.is_ge, op1=OP.mult,
                                               accum_out=zs[0:rc, :])
                nc.vector.reciprocal(rz[0:rc, :], zs[0:rc, :])
                nc.vector.tensor_scalar_mul(ar[0:rc, 0:N], ar[0:rc, 0:N],
                                            rz[0:rc, 0:1])

                at3d = AT[:].rearrange("p (f n) -> p f n", f=NCH)
                nc.sync.dma_start_transpose(out=at3d[:, :, r0:r0 + rc],
                                            in_=ar[0:rc, 0:NPAD])

        # A output: bf16 strips -> f32 rows (cast in SWDGE DMA)
        for fj, (r0, rc) in enumerate(CH):
            nc.gpsimd.dma_start(A_out[r0:r0 + rc, :], AT[0:rc, fj * N: fj * N + N])

        # ---------------- P3a: ctx, xx, LayerNorm ----------------
        with ExitStack() as p3:
            tmpb = p3.enter_context(tc.tile_pool(name="tmpb", bufs=1))
            tmpa = p3.enter_context(tc.tile_pool(name="tmpa", bufs=1))

            with ExitStack() as s1:
                ps_c = s1.enter_context(tc.tile_pool(name="ps_c", bufs=3, space="PSUM"))
                ctxT = tmpb.tile([H, N], f32, tag="ctxT")
                xxt = tmpb.tile([128, N], f32, tag="xxt")
                sqt = tmpa.tile([128, N], f32, tag="sqt")
                for o, w in _chunks512(N):
                    ps = ps_c.tile([H, 512], f32, tag="psc")
                    for fj, (jr0, jrc) in enumerate(CH):
                        nc.tensor.matmul(ps[0:H, 0:w],
                                         vnat[0:jrc, fj * H:(fj + 1) * H],
                                         AT[0:jrc, fj * N + o: fj * N + o + w],
                                         start=(fj == 0), stop=(fj == NCH - 1))
                    nc.scalar.activation(ctxT[0:H, o:o + w], ps[0:H, 0:w], AF.Copy)
                for o, w in _chunks512(N):
                    psx = ps_c.tile([128, 512], f32, tag="psx")
                    nc.tensor.matmul(psx[0:D, 0:w], sb["w_o"][:],
                                     ctxT[0:H, o:o + w], start=True, stop=True)
                    nc.scalar.activation(xxt[:, o:o + w], psx[0:D, 0:w], AF.Identity,
                                         bias=sb["bo"][:])
                    nc.vector.tensor_add(xxt[:, o:o + w], xxt[:, o:o + w],
                                         xsT[:, o:o + w])
                    nc.scalar.activation(sqt[:, o:o + w], xxt[:, o:o + w], AF.Square)

            with ExitStack() as s2:
                ps_l = s2.enter_context(tc.tile_pool(name="ps_l", bufs=1, space="PSUM"))
                psl = ps_l.tile([1, 2 * NPAD], f32, tag="psl")
                for o, w in _chunks512(N):
                    nc.tensor.matmul(psl[0:1, o:o + w], sb["ones128"][:],
                                     xxt[:, o:o + w], start=True, stop=True)
                    nc.tensor.matmul(psl[0:1, NPAD + o: NPAD + o + w],
                                     sb["ones128"][:], sqt[:, o:o + w],
                                     start=True, stop=True)
                # mu = colsum/128, msq = colsum(x^2)/128, isd = 1/sqrt(var+eps)
                lnv = tmpa.tile([1, 2 * NPAD], f32, tag="lnv")
                nc.vector.tensor_scalar(out=lnv[0:1, :], in0=psl[0:1, :],
                                        scalar1=1.0 / 128.0, scalar2=None,
                                        op0=OP.mult)
                lntmp = tmpa.tile([1, N], f32, tag="lntmp")
                nc.vector.tensor_mul(lntmp[0:1, :], lnv[0:1, 0:N], lnv[0:1, 0:N])
                nc.vector.tensor_sub(lnv[0:1, NPAD:NPAD + N],
                                     lnv[0:1, NPAD:NPAD + N], lntmp[0:1, :])
                nc.vector.tensor_scalar_add(lnv[0:1, NPAD:NPAD + N],
                                            lnv[0:1, NPAD:NPAD + N], 1e-6)
                nc.scalar.activation(lntmp[0:1, :], lnv[0:1, NPAD:NPAD + N],
                                     AF.Sqrt)
                nc.vector.reciprocal(lnv[0:1, NPAD:NPAD + N], lntmp[0:1, :])

                bc = tmpa.tile([128, 2 * NPAD], f32, tag="bc")
                nc.gpsimd.partition_broadcast(bc[0:128, 0:N], lnv[0:1, 0:N])
                nc.gpsimd.partition_broadcast(bc[0:128, NPAD:NPAD + N],
                                              lnv[0:1, NPAD:NPAD + N])
                nc.vector.tensor_sub(xxn[:, :], xxt[:, :], bc[:, 0:N])
                nc.vector.tensor_mul(xxn[:, :], xxn[:, :], bc[:, NPAD:NPAD + N])
                nc.vector.tensor_scalar(out=xxn[:, :], in0=xxn[:, :],
                                        scalar1=sb["lng"][:], scalar2=sb["lnb"][:],
                                        op0=OP.mult, op1=OP.add)

        # ---------------- P3b: gates and update ----------------
        with ExitStack() as p4:
            wrk = p4.enter_context(tc.tile_pool(name="wrk", bufs=1))

            # h12 = 0.5 * xxn @ [g1_W | g2_W]  (T-layout + natural bf16)
            h12T = wrk.tile([128, N], f32, tag="h12T")
            h12n = wrk.tile([128, NCH * D], bf16, tag="h12n")
            with ExitStack() as s3:
                ps_h = s3.enter_context(tc.tile_pool(name="ps_h", bufs=3, space="PSUM"))
                ps_t2 = s3.enter_context(tc.tile_pool(name="ps_t2", bufs=2, space="PSUM"))
                for o, w in _chunks512(N):
                    ps = ps_h.tile([128, 512], f32, tag="psh")
                    nc.tensor.matmul(ps[0:D, 0:w], sb["g12"][:],
                                     xxn[:, o:o + w], start=True, stop=True)
                    nc.scalar.activation(h12T[:, o:o + w], ps[0:D, 0:w],
                                         AF.Identity, scale=0.5)
                for fj, (r0, rc) in enumerate(CH):
                    pst = ps_t2.tile([128, D], f32, tag="pst2")
                    nc.tensor.transpose(pst[0:rc, 0:D], h12T[0:D, r0:r0 + rc],
                                        sb["ident"][:])
                    nc.scalar.activation(h12n[0:rc, fj * D:(fj + 1) * D],
                                         pst[0:rc, 0:D], AF.Copy)

            # zr = sigmoid(attn @ h12 + h12 + gb12); z/r as separate base-0 tiles
            zT = wrk.tile([H, N], f32, tag="zT")
            rT = wrk.tile([H, N], f32, tag="rT")
            scr = wrk.tile([128, N], f32, tag="scr")
            with ExitStack() as s4:
                ps_m = s4.enter_context(tc.tile_pool(name="ps_m", bufs=3, space="PSUM"))
                for o, w in _chunks512(N):
                    ps = ps_m.tile([128, 512], f32, tag="psm")
                    for fj, (jr0, jrc) in enumerate(CH):
                        nc.tensor.matmul(ps[0:D, 0:w],
                                         h12n[0:jrc, fj * D:(fj + 1) * D],
                                         AT[0:jrc, fj * N + o: fj * N + o + w],
                                         start=(fj == 0), stop=(fj == NCH - 1))
                    nc.vector.tensor_add(scr[:, o:o + w], ps[0:D, 0:w],
                                         h12T[:, o:o + w])
                    nc.scalar.activation(zT[0:H, o:o + w], scr[0:H, o:o + w],
                                         AF.Sigmoid, bias=sb["gb12"][0:H, :])
                    nc.scalar.activation(rT[0:H, o:o + w], scr[64:128, o:o + w],
                                         AF.Sigmoid, bias=sb["gb12"][64:128, :])

            # h3 = 0.5 * (xxn @ up1 + (z*state) @ up2)
            zst = wrk.tile([H, N], f32, tag="zst")
            nc.vector.tensor_mul(zst[0:H, :], zT[0:H, :], stT0[0:H, :])
            h3T = wrk.tile([H, N], f32, tag="h3T")
            h3n = wrk.tile([128, NCH * H], bf16, tag="h3n")
            with ExitStack() as s5:
                ps_3 = s5.enter_context(tc.tile_pool(name="ps_3", bufs=3, space="PSUM"))
                ps_t3 = s5.enter_context(tc.tile_pool(name="ps_t3", bufs=2, space="PSUM"))
                for o, w in _chunks512(N):
                    ps = ps_3.tile([H, 512], f32, tag="ps3")
                    nc.tensor.matmul(ps[0:H, 0:w], sb["up1"][:],
                                     xxn[:, o:o + w], start=True, stop=False)
                    nc.tensor.matmul(ps[0:H, 0:w], sb["up2"][:],
                                     zst[0:H, o:o + w], start=False, stop=True)
                    nc.scalar.activation(h3T[0:H, o:o + w], ps[0:H, 0:w],
                                         AF.Identity, scale=0.5)
                for fj, (r0, rc) in enumerate(CH):
                    pst = ps_t3.tile([128, H], f32, tag="pst3")
                    nc.tensor.transpose(pst[0:rc, 0:H], h3T[0:H, r0:r0 + rc],
                                        sb["ident"][0:H, 0:H])
                    nc.scalar.activation(h3n[0:rc, fj * H:(fj + 1) * H],
                                         pst[0:rc, 0:H], AF.Copy)

            # hc = tanh(attn @ h3 + h3 + up_b);  h = hc + r*(state - hc)
            hcT = wrk.tile([H, N], f32, tag="hcT")
            hT = wrk.tile([H, N], f32, tag="hT")
            with ExitStack() as s6:
                ps_m3 = s6.enter_context(tc.tile_pool(name="ps_m3", bufs=3, space="PSUM"))
                for o, w in _chunks512(N):
                    ps = ps_m3.tile([H, 512], f32, tag="psm3")
                    for fj, (jr0, jrc) in enumerate(CH):
                        nc.tensor.matmul(ps[0:H, 0:w],
                                         h3n[0:jrc, fj * H:(fj + 1) * H],
                                         AT[0:jrc, fj * N + o: fj * N + o + w],
                                         start=(fj == 0), stop=(fj == NCH - 1))
                    nc.vector.tensor_add(scr[0:H, o:o + w], ps[0:H, 0:w],
                                         h3T[0:H, o:o + w])
                    nc.scalar.activation(hcT[0:H, o:o + w], scr[0:H, o:o + w],
                                         AF.Tanh, bias=sb["upb"][:])
            nc.vector.tensor_sub(hT[0:H, :], stT0[0:H, :], hcT[0:H, :])
            nc.vector.tensor_mul(hT[0:H, :], hT[0:H, :], rT[0:H, :])
            nc.vector.tensor_add(hT[0:H, :], hT[0:H, :], hcT[0:H, :])

            # h output: transpose to natural layout and DMA out
            with ExitStack() as s7:
                ps_t4 = s7.enter_context(tc.tile_pool(name="ps_t4", bufs=2, space="PSUM"))
                hn = p4.enter_context(tc.tile_pool(name="hn", bufs=2))
                for fj, (r0, rc) in enumerate(CH):
                    pst = ps_t4.tile([128, H], f32, tag="pst4")
                    nc.tensor.transpose(pst[0:rc, 0:H], hT[0:H, r0:r0 + rc],
                                        sb["ident"][0:H, 0:H])
                    hnt = hn.tile([128, H], f32, tag="hnt")
                    nc.scalar.activation(hnt[0:rc, :], pst[0:rc, 0:H], AF.Copy)
                    nc.sync.dma_start(h_out[r0:r0 + rc, :], hnt[0:rc, :])

    nc.compile()
    return nc


def _prep_shards(inputs):
    g = lambda k: np.ascontiguousarray(np.asarray(inputs[k]), dtype=np.float32)
    x, state = g("x"), g("state")
    B = x.shape[0]
    assert x.shape == (B, N, DIN) and state.shape == (B, N, H)
    assert int(np.asarray(inputs["top_k"])) == K

    col = lambda a: np.asarray(a, np.float32).reshape(-1, 1)
    mats = {
        "lin_W": g("lin_W"), "w_q": g("Wq"), "w_k": g("Wk"), "w_v": g("Wv"),
        "w_o": g("Wo"),
        "g12": np.concatenate([g("g1_W"), g("g2_W")], axis=1),
        "up1": g("up_W")[:D], "up2": g("up_W")[D:],
        "linb": col(inputs["lin_b"]), "bq8": col(np.asarray(inputs["bq"]) / 8.0),
        "bk": col(inputs["bk"]), "bv": col(inputs["bv"]), "bo": col(inputs["bo"]),
        "gb12": col(np.concatenate([np.asarray(inputs["g1_b"]),
                                    np.asarray(inputs["g2_b"])])),
        "upb": col(inputs["up_b"]),
        "lng": col(inputs["ln_g"]), "lnb": col(inputs["ln_b"]),
        "ident": np.eye(128, dtype=np.float32),
        "ones128": np.ones((128, 1), dtype=np.float32),
    }
    wpack = np.zeros((128, WPACK), dtype=np.float32)
    for name, (p, c0, w) in WLAYOUT.items():
        a = mats[name]
        assert a.shape == (p, w), (name, a.shape, (p, w))
        wpack[0:p, c0:c0 + w] = a
    common = {"wpack": wpack}
    in_maps = []
    for c in range(B):
        m = dict(common)
        m["xT"] = np.ascontiguousarray(x[c].T)
        m["stT"] = np.ascontiguousarray(state[c].T)
        in_maps.append(m)
    return in_maps, B


def kernel(**inputs):
    global _PROG
    from concourse.bass_utils import run_bass_kernel_spmd

    if _PROG is None:
        _PROG = _build_program()
    nc = _PROG

    in_maps, B = _prep_shards(inputs)
    assert B == 8, f"expected B=8, got {B}"
    res = run_bass_kernel_spmd(nc, in_maps, core_ids=list(range(B)))
    h = np.stack([res.results[c]["h_out"] for c in range(B)])
    A = np.stack([res.results[c]["A_out"] for c in range(B)])
    return h, A


# revision 16
# speedup vs baseline: 1.4367x; 1.1978x over previous
"""Trainium2 Bass kernel for nn_AGRNNCell (attention + top-k + GCN-gated GRU cell).

Sharding: batch dim B=8 across 8 NeuronCores (one graph per core); the small
weight matrices are replicated.

Per-core algorithm (N=2000 nodes, H=64, D=128, K=20). Most tensors are kept in
transposed "feature-major" layout [feat, node]:

  xsT   = [lin_W^T @ xT + lin_b ; stateT]                      [128, N]
  qsT   = (Wq^T @ xsT + bq) / 8,  kT = Wk^T @ xsT + bk         [64, N]
  S_ci  = qsT[:,chunk]^T @ kT  (f32 PSUM, 16 row-chunks)       [128, N]
  top-k: 3x max8 + 2x match_replace -> rank-20 value t_p per row; Z = sum of
         exp over the kept 20 values (shift-free exp; scores are O(1)).
  A_row = exp(S) * (S >= t) / Z   (attn, bf16)
  AT    = xbar-DMA transpose of A_row (bf16) = attn^T; also the A output
          (cast bf16->f32 during the output DMA).
  Dense-GCN trick: the masked softmax has exactly 20 nonzeros per row summing
  to 1, so GCNConv's degree is exactly 2 (+O(1e-7)) and
  prop(h) = dinv*(Attn+I)(dinv*h) = 0.5*(attn @ h + h).
  ctx^T = v^T-stationary matmuls streaming AT
  xx^T  = Wo^T ctx^T + bo + xsT; LayerNorm over features via PE column-sums
  zr^T  = sigmoid(m12 + h12 + [g1_b;g2_b]),  h12 = 0.5 * xxn @ [g1_W|g2_W],
          m12 = attn @ h12  (streams AT with h12 in natural layout)
  hc^T  = tanh(m3 + h3 + up_b),  h3 = 0.5 * (xxn @ up1 + (z*state) @ up2)
  h     = hc + r*(state - hc)

kernel(**inputs) takes FULL unsharded inputs, returns (h, A) like reference.
"""

import numpy as np

N = 2000
DIN = 32
H = 64
D = 128
K = 20
NCH = 16  # row chunks of 128 (last is 80)
CH = [(ci * 128, min(128, N - ci * 128)) for ci in range(NCH)]
NPAD = 2048  # A_row free-dim pad so every transpose block is 128 wide

# packed weights layout: name -> (partitions, col offset, col width)
WLAYOUT = {
    "lin_W": (32, 0, 64), "w_q": (128, 64, 64), "w_k": (128, 128, 64),
    "w_v": (128, 192, 64), "w_o": (64, 256, 128), "g12": (128, 384, 128),
    "up1": (128, 512, 64), "up2": (64, 576, 64), "ident": (128, 640, 128),
    "linb": (64, 768, 1), "bq8": (64, 769, 1), "bk": (64, 770, 1),
    "bv": (64, 771, 1), "bo": (128, 772, 1), "gb12": (128, 773, 1),
    "upb": (64, 774, 1), "lng": (128, 775, 1), "lnb": (128, 776, 1),
    "ones128": (128, 777, 1),
}
WPACK = 784

_PROG = None


def _chunks512(width):
    out, o = [], 0
    while o < width:
        w = min(512, width - o)
        out.append((o, w))
        o += w
    return out


def _build_program():
    from contextlib import ExitStack

    import concourse.bacc as bacc
    import concourse.mybir as mybir
    import concourse.tile as tile

    f32 = mybir.dt.float32
    bf16 = mybir.dt.bfloat16
    AF = mybir.ActivationFunctionType
    OP = mybir.AluOpType

    nc = bacc.Bacc("TRN2", target_bir_lowering=False, debug=False, num_devices=8)

    din = {}
    for name, shape in [
        ("xT", [DIN, N]), ("stT", [H, N]), ("wpack", [128, WPACK]),
    ]:
        din[name] = nc.dram_tensor(name, shape, f32, kind="ExternalInput").ap()
    h_out = nc.dram_tensor("h_out", [N, H], f32, kind="ExternalOutput").ap()
    A_out = nc.dram_tensor("A_out", [N, N], f32, kind="ExternalOutput").ap()

    with tile.TileContext(nc) as tc, ExitStack() as top:
        # ---------------- persistent SBUF ----------------
        pers = top.enter_context(tc.tile_pool(name="pers", bufs=1))
        wk = pers.tile([128, WPACK], f32, tag="wpack")
        nc.sync.dma_start(wk[:], din["wpack"])
        sb = {name: wk[0:p, c0:c0 + w] for name, (p, c0, w) in WLAYOUT.items()}

        xs0 = pers.tile([128, N], f32, tag="xs0")  # [0:32]=xT, [64:128]=stateT
        nc.sync.dma_start(xs0[0:DIN, :], din["xT"])
        nc.sync.dma_start(xs0[64:128, :], din["stT"])
        stT0 = pers.tile([H, N], f32, tag="stT0")  # base-0 copy for DVE ops
        nc.sync.dma_start(stT0[0:H, :], din["stT"])

        xsT = pers.tile([128, N], f32, tag="xsT")
        qsT = pers.tile([H, N], f32, tag="qsT")
        kT = pers.tile([H, N], f32, tag="kT")
        vnat = pers.tile([128, NCH * H], bf16, tag="vnat")
        AT = pers.tile([128, NCH * N], bf16, tag="AT")  # strip fj = [:, fj*N:(fj+1)*N]
        xxn = pers.tile([128, N], f32, tag="xxn")

        # ---------------- P1: projections ----------------
        with ExitStack() as p1:
            ps_a = p1.enter_context(tc.tile_pool(name="ps_a", bufs=4, space="PSUM"))
            ps_tr = p1.enter_context(tc.tile_pool(name="ps_tr", bufs=2, space="PSUM"))
            tmp1 = p1.enter_context(tc.tile_pool(name="tmp1", bufs=1))

            for o, w in _chunks512(N):
                psq = ps_a.tile([H, 512], f32, tag="ps1")
                nc.tensor.matmul(psq[0:H, 0:w], sb["lin_W"][:],
                                 xs0[0:DIN, o:o + w], start=True, stop=True)
                nc.scalar.activation(xsT[0:H, o:o + w], psq[0:H, 0:w], AF.Identity,
                                     bias=sb["linb"][:])
            nc.vector.tensor_copy(xsT[64:128, :], xs0[64:128, :])

            vT = tmp1.tile([H, N], f32, tag="vT")
            for wname, bname, dst, scale in [("w_q", "bq8", qsT, 0.125),
                                             ("w_k", "bk", kT, 1.0),
                                             ("w_v", "bv", vT, 1.0)]:
                for o, w in _chunks512(N):
                    psq = ps_a.tile([H, 512], f32, tag="ps1")
                    nc.tensor.matmul(psq[0:H, 0:w], sb[wname][:],
                                     xsT[:, o:o + w], start=True, stop=True)
                    nc.scalar.activation(dst[0:H, o:o + w], psq[0:H, 0:w],
                                         AF.Identity, bias=sb[bname][:], scale=scale)

            for fj, (r0, rc) in enumerate(CH):
                pst = ps_tr.tile([128, H], f32, tag="pst")
                nc.tensor.transpose(pst[0:rc, 0:H], vT[0:H, r0:r0 + rc],
                                    sb["ident"][0:H, 0:H])
                nc.scalar.activation(vnat[0:rc, fj * H:(fj + 1) * H],
                                     pst[0:rc, 0:H], AF.Copy)

        # -------- P2: scores, top-k, attention, transpose --------
        with ExitStack() as p2:
            ps_s = p2.enter_context(tc.tile_pool(name="ps_s", bufs=2, space="PSUM"))
            scw = p2.enter_context(tc.tile_pool(name="scw", bufs=2))
            vps = p2.enter_context(tc.tile_pool(name="vps", bufs=2))
            epool = p2.enter_context(tc.tile_pool(name="epool", bufs=2))
            apool = p2.enter_context(tc.tile_pool(name="apool", bufs=3))

            for ci, (r0, rc) in enumerate(CH):
                ps = ps_s.tile([128, NPAD], f32, tag="S")
                for o, w in _chunks512(N):
                    nc.tensor.matmul(ps[0:rc, o:o + w], qsT[0:H, r0:r0 + rc],
                                     kT[0:H, o:o + w], start=True, stop=True)

                vv = vps.tile([128, 24], f32, tag="vv")
                scp = scw.tile([128, N], f32, tag="scp")
                nc.vector.max(out=vv[0:rc, 0:8], in_=ps[0:rc, 0:N])
                nc.vector.match_replace(out=scp[0:rc, :], in_to_replace=vv[0:rc, 0:8],
                                        in_values=ps[0:rc, 0:N], imm_value=-1e30)
                nc.vector.max(out=vv[0:rc, 8:16], in_=scp[0:rc, :])
                nc.vector.match_replace(out=scp[0:rc, :], in_to_replace=vv[0:rc, 8:16],
                                        in_values=scp[0:rc, :], imm_value=-1e30)
                nc.vector.max(out=vv[0:rc, 16:24], in_=scp[0:rc, :])

                # E = exp(S) (ACT, overlaps max8 rounds; last PSUM reader).
                # Mask on E itself: S >= t  <=>  E >= exp(t) (exp monotone;
                # LUT collisions at the boundary are ~1e-7-probability).
                # Fused A_unnorm = (E >= exp(t)) * E with accum Z = rowsum,
                # then A_row = A_unnorm / Z (bf16 4x pass).
                et = epool.tile([128, N], f32, tag="E")
                nc.scalar.activation(et[0:rc, :], ps[0:rc, 0:N], AF.Exp)
                et20 = vps.tile([128, 1], f32, tag="et20")
                nc.scalar.activation(et20[0:rc, :], vv[0:rc, 19:20], AF.Exp)
                zs = vps.tile([128, 1], f32, tag="zs")
                rz = vps.tile([128, 1], f32, tag="rz")
                ar = apool.tile([128, NPAD], bf16, tag="ar")
                nc.vector.scalar_tensor_tensor(out=ar[0:rc, 0:N], in0=et[0:rc, :],
                                               scalar=et20[0:rc, 0:1],
                                               in1=et[0:rc, :],
                                               op0=OP.is_ge, op1=OP.mult,
                                               accum_out=zs[0:rc, :])
                nc.vector.reciprocal(rz[0:rc, :], zs[0:rc, :])
                nc.vector.tensor_scalar_mul(ar[0:rc, 0:N], ar[0:rc, 0:N],
                                            rz[0:rc, 0:1])

                at3d = AT[:].rearrange("p (f n) -> p f n", f=NCH)
                nc.sync.dma_start_transpose(out=at3d[:, :, r0:r0 + rc],
                                            in_=ar[0:rc, 0:NPAD])

        # A output: bf16 strips -> f32 rows (cast in SWDGE DMA)
        for fj, (r0, rc) in enumerate(CH):
            nc.gpsimd.dma_start(A_out[r0:r0 + rc, :], AT[0:rc, fj * N: fj * N + N])

        # ---------------- P3a: ctx, xx, LayerNorm ----------------
        with ExitStack() as p3:
            tmpb = p3.enter_context(tc.tile_pool(name="tmpb", bufs=1))
            tmpa = p3.enter_context(tc.tile_pool(name="tmpa", bufs=1))

            with ExitStack() as s1:
                ps_c = s1.enter_context(tc.tile_pool(name="ps_c", bufs=3, space="PSUM"))
                ctxT = tmpb.tile([H, N], f32, tag="ctxT")
                xxt = tmpb.tile([128, N], f32, tag="xxt")
                sqt = tmpa.tile([128, N], f32, tag="sqt")
                for o, w in _chunks512(N):
                    ps = ps_c.tile([H, 512], f32, tag="psc")
                    for fj, (jr0, jrc) in enumerate(CH):
                        nc.tensor.matmul(ps[0:H, 0:w],
                                         vnat[0:jrc, fj * H:(fj + 1) * H],
                                         AT[0:jrc, fj * N + o: fj * N + o + w],
                                         start=(fj == 0), stop=(fj == NCH - 1))
                    nc.scalar.activation(ctxT[0:H, o:o + w], ps[0:H, 0:w], AF.Copy)
                for o, w in _chunks512(N):
                    psx = ps_c.tile([128, 512], f32, tag="psx")
                    nc.tensor.matmul(psx[0:D, 0:w], sb["w_o"][:],
                                     ctxT[0:H, o:o + w], start=True, stop=True)
                    nc.scalar.activation(xxt[:, o:o + w], psx[0:D, 0:w], AF.Identity,
                                         bias=sb["bo"][:])
                    nc.vector.tensor_add(xxt[:, o:o + w], xxt[:, o:o + w],
                                         xsT[:, o:o + w])
                    nc.scalar.activation(sqt[:, o:o + w], xxt[:, o:o + w], AF.Square)

            with ExitStack() as s2:
                ps_l = s2.enter_context(tc.tile_pool(name="ps_l", bufs=1, space="PSUM"))
                psl = ps_l.tile([1, 2 * NPAD], f32, tag="psl")
                for o, w in _chunks512(N):
                    nc.tensor.matmul(psl[0:1, o:o + w], sb["ones128"][:],
                                     xxt[:, o:o + w], start=True, stop=True)
                    nc.tensor.matmul(psl[0:1, NPAD + o: NPAD + o + w],
                                     sb["ones128"][:], sqt[:, o:o + w],
                                     start=True, stop=True)
                # mu = colsum/128, msq = colsum(x^2)/128, isd = 1/sqrt(var+eps)
                lnv = tmpa.tile([1, 2 * NPAD], f32, tag="lnv")
                nc.vector.tensor_scalar(out=lnv[0:1, :], in0=psl[0:1, :],
                                        scalar1=1.0 / 128.0, scalar2=None,
                                        op0=OP.mult)
                lntmp = tmpa.tile([1, N], f32, tag="lntmp")
                nc.vector.tensor_mul(lntmp[0:1, :], lnv[0:1, 0:N], lnv[0:1, 0:N])
                nc.vector.tensor_sub(lnv[0:1, NPAD:NPAD + N],
                                     lnv[0:1, NPAD:NPAD + N], lntmp[0:1, :])
                nc.vector.tensor_scalar_add(lnv[0:1, NPAD:NPAD + N],
                                            lnv[0:1, NPAD:NPAD + N], 1e-6)
                # isd = 1/sqrt(var+eps) in one LUT (DVE reciprocal on a
                # 1-partition 2000-vector costs 8 cyc/elem = ~16us)
                nc.scalar.activation(lntmp[0:1, :], lnv[0:1, NPAD:NPAD + N],
                                     AF.Abs_reciprocal_sqrt)
                nc.vector.tensor_copy(lnv[0:1, NPAD:NPAD + N], lntmp[0:1, :])

                bc = tmpa.tile([128, 2 * NPAD], f32, tag="bc")
                nc.gpsimd.partition_broadcast(bc[0:128, 0:N], lnv[0:1, 0:N])
                nc.gpsimd.partition_broadcast(bc[0:128, NPAD:NPAD + N],
                                              lnv[0:1, NPAD:NPAD + N])
                nc.vector.tensor_sub(xxn[:, :], xxt[:, :], bc[:, 0:N])
                nc.vector.tensor_mul(xxn[:, :], xxn[:, :], bc[:, NPAD:NPAD + N])
                nc.vector.tensor_scalar(out=xxn[:, :], in0=xxn[:, :],
                                        scalar1=sb["lng"][:], scalar2=sb["lnb"][:],
                                        op0=OP.mult, op1=OP.add)

        # ---------------- P3b: gates and update ----------------
        with ExitStack() as p4:
            wrk = p4.enter_context(tc.tile_pool(name="wrk", bufs=1))

            # h12 = 0.5 * xxn @ [g1_W | g2_W]  (T-layout + natural bf16)
            h12T = wrk.tile([128, N], f32, tag="h12T")
            h12n = wrk.tile([128, NCH * D], bf16, tag="h12n")
            with ExitStack() as s3:
                ps_h = s3.enter_context(tc.tile_pool(name="ps_h", bufs=3, space="PSUM"))
                ps_t2 = s3.enter_context(tc.tile_pool(name="ps_t2", bufs=2, space="PSUM"))
                for o, w in _chunks512(N):
                    ps = ps_h.tile([128, 512], f32, tag="psh")
                    nc.tensor.matmul(ps[0:D, 0:w], sb["g12"][:],
                                     xxn[:, o:o + w], start=True, stop=True)
                    nc.scalar.activation(h12T[:, o:o + w], ps[0:D, 0:w],
                                         AF.Identity, scale=0.5)
                for fj, (r0, rc) in enumerate(CH):
                    pst = ps_t2.tile([128, D], f32, tag="pst2")
                    nc.tensor.transpose(pst[0:rc, 0:D], h12T[0:D, r0:r0 + rc],
                                        sb["ident"][:])
                    nc.scalar.activation(h12n[0:rc, fj * D:(fj + 1) * D],
                                         pst[0:rc, 0:D], AF.Copy)

            # zr = sigmoid(attn @ h12 + h12 + gb12); z/r as separate base-0 tiles
            zT = wrk.tile([H, N], f32, tag="zT")
            rT = wrk.tile([H, N], f32, tag="rT")
            scr = wrk.tile([128, N], f32, tag="scr")
            with ExitStack() as s4:
                ps_m = s4.enter_context(tc.tile_pool(name="ps_m", bufs=3, space="PSUM"))
                for o, w in _chunks512(N):
                    ps = ps_m.tile([128, 512], f32, tag="psm")
                    for fj, (jr0, jrc) in enumerate(CH):
                        nc.tensor.matmul(ps[0:D, 0:w],
                                         h12n[0:jrc, fj * D:(fj + 1) * D],
                                         AT[0:jrc, fj * N + o: fj * N + o + w],
                                         start=(fj == 0), stop=(fj == NCH - 1))
                    nc.vector.tensor_add(scr[:, o:o + w], ps[0:D, 0:w],
                                         h12T[:, o:o + w])
                    nc.scalar.activation(zT[0:H, o:o + w], scr[0:H, o:o + w],
                                         AF.Sigmoid, bias=sb["gb12"][0:H, :])
                    nc.scalar.activation(rT[0:H, o:o + w], scr[64:128, o:o + w],
                                         AF.Sigmoid, bias=sb["gb12"][64:128, :])

            # h3 = 0.5 * (xxn @ up1 + (z*state) @ up2)
            zst = wrk.tile([H, N], f32, tag="zst")
            nc.vector.tensor_mul(zst[0:H, :], zT[0:H, :], stT0[0:H, :])
            h3T = wrk.tile([H, N], f32, tag="h3T")
            h3n = wrk.tile([128, NCH * H], bf16, tag="h3n")
            with ExitStack() as s5:
                ps_3 = s5.enter_context(tc.tile_pool(name="ps_3", bufs=3, space="PSUM"))
                ps_t3 = s5.enter_context(tc.tile_pool(name="ps_t3", bufs=2, space="PSUM"))
                for o, w in _chunks512(N):
                    ps = ps_3.tile([H, 512], f32, tag="ps3")
                    nc.tensor.matmul(ps[0:H, 0:w], sb["up1"][:],
                                     xxn[:, o:o + w], start=True, stop=False)
                    nc.tensor.matmul(ps[0:H, 0:w], sb["up2"][:],
                                     zst[0:H, o:o + w], start=False, stop=True)
                    nc.scalar.activation(h3T[0:H, o:o + w], ps[0:H, 0:w],
                                         AF.Identity, scale=0.5)
                for fj, (r0, rc) in enumerate(CH):
                    pst = ps_t3.tile([128, H], f32, tag="pst3")
                    nc.tensor.transpose(pst[0:rc, 0:H], h3T[0:H, r0:r0 + rc],
                                        sb["ident"][0:H, 0:H])
                    nc.scalar.activation(h3n[0:rc, fj * H:(fj + 1) * H],
                                         pst[0:rc, 0:H], AF.Copy)

            # hc = tanh(attn @ h3 + h3 + up_b);  h = hc + r*(state - hc)
            hcT = wrk.tile([H, N], f32, tag="hcT")
            hT = wrk.tile([H, N], f32, tag="hT")
            with ExitStack() as s6:
                ps_m3 = s6.enter_context(tc.tile_pool(name="ps_m3", bufs=3, space="PSUM"))
                for o, w in _chunks512(N):
                    ps = ps_m3.tile([H, 512], f32, tag="psm3")
                    for fj, (jr0, jrc) in enumerate(CH):
                        nc.tensor.matmul(ps[0:H, 0:w],
                                         h3n[0:jrc, fj * H:(fj + 1) * H],
                                         AT[0:jrc, fj * N + o: fj * N + o + w],
                                         start=(fj == 0), stop=(fj == NCH - 1))
                    nc.vector.tensor_add(scr[0:H, o:o + w], ps[0:H, 0:w],
                                         h3T[0:H, o:o + w])
                    nc.scalar.activation(hcT[0:H, o:o + w], scr[0:H, o:o + w],
                                         AF.Tanh, bias=sb["upb"][:])
            nc.vector.tensor_sub(hT[0:H, :], stT0[0:H, :], hcT[0:H, :])
            nc.vector.tensor_mul(hT[0:H, :], hT[0:H, :], rT[0:H, :])
            nc.vector.tensor_add(hT[0:H, :], hT[0:H, :], hcT[0:H, :])

            # h output: transpose to natural layout and DMA out
            with ExitStack() as s7:
                ps_t4 = s7.enter_context(tc.tile_pool(name="ps_t4", bufs=2, space="PSUM"))
                hn = p4.enter_context(tc.tile_pool(name="hn", bufs=2))
                for fj, (r0, rc) in enumerate(CH):
                    pst = ps_t4.tile([128, H], f32, tag="pst4")
                    nc.tensor.transpose(pst[0:rc, 0:H], hT[0:H, r0:r0 + rc],
                                        sb["ident"][0:H, 0:H])
                    hnt = hn.tile([128, H], f32, tag="hnt")
                    nc.scalar.activation(hnt[0:rc, :], pst[0:rc, 0:H], AF.Copy)
                    nc.sync.dma_start(h_out[r0:r0 + rc, :], hnt[0:rc, :])

    nc.compile()
    return nc


def _prep_shards(inputs):
    g = lambda k: np.ascontiguousarray(np.asarray(inputs[k]), dtype=np.float32)
    x, state = g("x"), g("state")
    B = x.shape[0]
    assert x.shape == (B, N, DIN) and state.shape == (B, N, H)
    assert int(np.asarray(inputs["top_k"])) == K

    col = lambda a: np.asarray(a, np.float32).reshape(-1, 1)
    mats = {
        "lin_W": g("lin_W"), "w_q": g("Wq"), "w_k": g("Wk"), "w_v": g("Wv"),
        "w_o": g("Wo"),
        "g12": np.concatenate([g("g1_W"), g("g2_W")], axis=1),
        "up1": g("up_W")[:D], "up2": g("up_W")[D:],
        "linb": col(inputs["lin_b"]), "bq8": col(np.asarray(inputs["bq"]) / 8.0),
        "bk": col(inputs["bk"]), "bv": col(inputs["bv"]), "bo": col(inputs["bo"]),
        "gb12": col(np.concatenate([np.asarray(inputs["g1_b"]),
                                    np.asarray(inputs["g2_b"])])),
        "upb": col(inputs["up_b"]),
        "lng": col(inputs["ln_g"]), "lnb": col(inputs["ln_b"]),
        "ident": np.eye(128, dtype=np.float32),
        "ones128": np.ones((128, 1), dtype=np.float32),
    }
    wpack = np.zeros((128, WPACK), dtype=np.float32)
    for name, (p, c0, w) in WLAYOUT.items():
        a = mats[name]
        assert a.shape == (p, w), (name, a.shape, (p, w))
        wpack[0:p, c0:c0 + w] = a
    common = {"wpack": wpack}
    in_maps = []
    for c in range(B):
        m = dict(common)
        m["xT"] = np.ascontiguousarray(x[c].T)
        m["stT"] = np.ascontiguousarray(state[c].T)
        in_maps.append(m)
    return in_maps, B


def kernel(**inputs):
    global _PROG
    from concourse.bass_utils import run_bass_kernel_spmd

    if _PROG is None:
        _PROG = _build_program()
    nc = _PROG

    in_maps, B = _prep_shards(inputs)
    assert B == 8, f"expected B=8, got {B}"
    res = run_bass_kernel_spmd(nc, in_maps, core_ids=list(range(B)))
    h = np.stack([res.results[c]["h_out"] for c in range(B)])
    A = np.stack([res.results[c]["A_out"] for c in range(B)])
    return h, A


# revision 17
# speedup vs baseline: 1.4746x; 1.0264x over previous
"""Trainium2 Bass kernel for nn_AGRNNCell (attention + top-k + GCN-gated GRU cell).

Sharding: batch dim B=8 across 8 NeuronCores (one graph per core); the small
weight matrices are replicated.

Per-core algorithm (N=2000 nodes, H=64, D=128, K=20). Most tensors are kept in
transposed "feature-major" layout [feat, node]:

  xsT   = [lin_W^T @ xT + lin_b ; stateT]                      [128, N]
  qsT   = (Wq^T @ xsT + bq) / 8,  kT = Wk^T @ xsT + bk         [64, N]
  S_ci  = qsT[:,chunk]^T @ kT  (f32 PSUM, 16 row-chunks)       [128, N]
  top-k: 3x max8 + 2x match_replace -> rank-20 value t_p per row; Z = sum of
         exp over the kept 20 values (shift-free exp; scores are O(1)).
  A_row = exp(S) * (S >= t) / Z   (attn, bf16)
  AT    = xbar-DMA transpose of A_row (bf16) = attn^T; also the A output
          (cast bf16->f32 during the output DMA).
  Dense-GCN trick: the masked softmax has exactly 20 nonzeros per row summing
  to 1, so GCNConv's degree is exactly 2 (+O(1e-7)) and
  prop(h) = dinv*(Attn+I)(dinv*h) = 0.5*(attn @ h + h).
  ctx^T = v^T-stationary matmuls streaming AT
  xx^T  = Wo^T ctx^T + bo + xsT; LayerNorm over features via PE column-sums
  zr^T  = sigmoid(m12 + h12 + [g1_b;g2_b]),  h12 = 0.5 * xxn @ [g1_W|g2_W],
          m12 = attn @ h12  (streams AT with h12 in natural layout)
  hc^T  = tanh(m3 + h3 + up_b),  h3 = 0.5 * (xxn @ up1 + (z*state) @ up2)
  h     = hc + r*(state - hc)

kernel(**inputs) takes FULL unsharded inputs, returns (h, A) like reference.
"""

import numpy as np

N = 2000
DIN = 32
H = 64
D = 128
K = 20
NCH = 16  # row chunks of 128 (last is 80)
CH = [(ci * 128, min(128, N - ci * 128)) for ci in range(NCH)]
NPAD = 2048  # A_row free-dim pad so every transpose block is 128 wide

# packed weights layout: name -> (partitions, col offset, col width)
WLAYOUT = {
    "lin_W": (32, 0, 64), "w_q": (128, 64, 64), "w_k": (128, 128, 64),
    "w_v": (128, 192, 64), "w_o": (64, 256, 128), "g12": (128, 384, 128),
    "up1": (128, 512, 64), "up2": (64, 576, 64), "ident": (128, 640, 128),
    "linb": (64, 768, 1), "bq8": (64, 769, 1), "bk": (64, 770, 1),
    "bv": (64, 771, 1), "bo": (128, 772, 1), "gb12": (128, 773, 1),
    "upb": (64, 774, 1), "lng": (128, 775, 1), "lnb": (128, 776, 1),
    "ones128": (128, 777, 1),
}
WPACK = 784

_PROG = None


def _chunks512(width):
    out, o = [], 0
    while o < width:
        w = min(512, width - o)
        out.append((o, w))
        o += w
    return out


def _build_program():
    from contextlib import ExitStack

    import concourse.bacc as bacc
    import concourse.mybir as mybir
    import concourse.tile as tile

    f32 = mybir.dt.float32
    bf16 = mybir.dt.bfloat16
    AF = mybir.ActivationFunctionType
    OP = mybir.AluOpType

    nc = bacc.Bacc("TRN2", target_bir_lowering=False, debug=False, num_devices=8)

    din = {}
    for name, shape in [
        ("xT", [DIN, N]), ("stT", [H, N]), ("wpack", [128, WPACK]),
    ]:
        din[name] = nc.dram_tensor(name, shape, f32, kind="ExternalInput").ap()
    h_out = nc.dram_tensor("h_out", [N, H], f32, kind="ExternalOutput").ap()
    A_out = nc.dram_tensor("A_out", [N, N], f32, kind="ExternalOutput").ap()

    with tile.TileContext(nc) as tc, ExitStack() as top:
        # ---------------- persistent SBUF ----------------
        pers = top.enter_context(tc.tile_pool(name="pers", bufs=1))
        wk = pers.tile([128, WPACK], f32, tag="wpack")
        nc.sync.dma_start(wk[:], din["wpack"])
        sb = {name: wk[0:p, c0:c0 + w] for name, (p, c0, w) in WLAYOUT.items()}

        xs0 = pers.tile([128, N], f32, tag="xs0")  # [0:32]=xT, [64:128]=stateT
        nc.sync.dma_start(xs0[0:DIN, :], din["xT"])
        nc.sync.dma_start(xs0[64:128, :], din["stT"])
        stT0 = pers.tile([H, N], f32, tag="stT0")  # base-0 copy for DVE ops
        nc.sync.dma_start(stT0[0:H, :], din["stT"])

        xsT = pers.tile([128, N], f32, tag="xsT")
        qsT = pers.tile([H, N], f32, tag="qsT")
        kT = pers.tile([H, N], f32, tag="kT")
        vnat = pers.tile([128, NCH * H], bf16, tag="vnat")
        AT = pers.tile([128, NCH * N], bf16, tag="AT")  # strip fj = [:, fj*N:(fj+1)*N]
        xxn = pers.tile([128, N], f32, tag="xxn")

        # ---------------- P1: projections ----------------
        with ExitStack() as p1:
            ps_a = p1.enter_context(tc.tile_pool(name="ps_a", bufs=4, space="PSUM"))
            ps_tr = p1.enter_context(tc.tile_pool(name="ps_tr", bufs=2, space="PSUM"))
            tmp1 = p1.enter_context(tc.tile_pool(name="tmp1", bufs=1))

            for o, w in _chunks512(N):
                psq = ps_a.tile([H, 512], f32, tag="ps1")
                nc.tensor.matmul(psq[0:H, 0:w], sb["lin_W"][:],
                                 xs0[0:DIN, o:o + w], start=True, stop=True)
                nc.scalar.activation(xsT[0:H, o:o + w], psq[0:H, 0:w], AF.Identity,
                                     bias=sb["linb"][:])
            nc.vector.tensor_copy(xsT[64:128, :], xs0[64:128, :])

            vTb = tmp1.tile([H, NPAD], bf16, tag="vTb")
            for wname, bname, dst, scale in [("w_q", "bq8", qsT, 0.125),
                                             ("w_k", "bk", kT, 1.0),
                                             ("w_v", "bv", vTb, 1.0)]:
                for o, w in _chunks512(N):
                    psq = ps_a.tile([H, 512], f32, tag="ps1")
                    nc.tensor.matmul(psq[0:H, 0:w], sb[wname][:],
                                     xsT[:, o:o + w], start=True, stop=True)
                    nc.scalar.activation(dst[0:H, o:o + w], psq[0:H, 0:w],
                                         AF.Identity, bias=sb[bname][:], scale=scale)

            vn3 = vnat[:].rearrange("p (f r) -> p f r", f=NCH)
            nc.sync.dma_start_transpose(out=vn3, in_=vTb[0:H, 0:NPAD])

        # -------- P2: scores, top-k, attention, transpose --------
        with ExitStack() as p2:
            ps_s = p2.enter_context(tc.tile_pool(name="ps_s", bufs=2, space="PSUM"))
            scw = p2.enter_context(tc.tile_pool(name="scw", bufs=2))
            vps = p2.enter_context(tc.tile_pool(name="vps", bufs=2))
            epool = p2.enter_context(tc.tile_pool(name="epool", bufs=2))
            apool = p2.enter_context(tc.tile_pool(name="apool", bufs=3))

            for ci, (r0, rc) in enumerate(CH):
                ps = ps_s.tile([128, NPAD], f32, tag="S")
                for o, w in _chunks512(N):
                    nc.tensor.matmul(ps[0:rc, o:o + w], qsT[0:H, r0:r0 + rc],
                                     kT[0:H, o:o + w], start=True, stop=True)

                vv = vps.tile([128, 24], f32, tag="vv")
                scp = scw.tile([128, N], f32, tag="scp")
                nc.vector.max(out=vv[0:rc, 0:8], in_=ps[0:rc, 0:N])
                nc.vector.match_replace(out=scp[0:rc, :], in_to_replace=vv[0:rc, 0:8],
                                        in_values=ps[0:rc, 0:N], imm_value=-1e30)
                nc.vector.max(out=vv[0:rc, 8:16], in_=scp[0:rc, :])
                nc.vector.match_replace(out=scp[0:rc, :], in_to_replace=vv[0:rc, 8:16],
                                        in_values=scp[0:rc, :], imm_value=-1e30)
                nc.vector.max(out=vv[0:rc, 16:24], in_=scp[0:rc, :])

                # E = exp(S) (ACT, overlaps max8 rounds; last PSUM reader).
                # Mask on E itself: S >= t  <=>  E >= exp(t) (exp monotone;
                # LUT collisions at the boundary are ~1e-7-probability).
                # Fused A_unnorm = (E >= exp(t)) * E with accum Z = rowsum,
                # then A_row = A_unnorm / Z (bf16 4x pass).
                et = epool.tile([128, N], f32, tag="E")
                nc.scalar.activation(et[0:rc, :], ps[0:rc, 0:N], AF.Exp)
                et20 = vps.tile([128, 1], f32, tag="et20")
                nc.scalar.activation(et20[0:rc, :], vv[0:rc, 19:20], AF.Exp)
                zs = vps.tile([128, 1], f32, tag="zs")
                rz = vps.tile([128, 1], f32, tag="rz")
                ar = apool.tile([128, NPAD], bf16, tag="ar")
                nc.vector.scalar_tensor_tensor(out=ar[0:rc, 0:N], in0=et[0:rc, :],
                                               scalar=et20[0:rc, 0:1],
                                               in1=et[0:rc, :],
                                               op0=OP.is_ge, op1=OP.mult,
                                               accum_out=zs[0:rc, :])
                nc.vector.reciprocal(rz[0:rc, :], zs[0:rc, :])
                nc.vector.tensor_scalar_mul(ar[0:rc, 0:N], ar[0:rc, 0:N],
                                            rz[0:rc, 0:1])

                at3d = AT[:].rearrange("p (f n) -> p f n", f=NCH)
                nc.sync.dma_start_transpose(out=at3d[:, :, r0:r0 + rc],
                                            in_=ar[0:rc, 0:NPAD])

        # A output: bf16 strips -> f32 rows (cast in SWDGE DMA)
        for fj, (r0, rc) in enumerate(CH):
            nc.gpsimd.dma_start(A_out[r0:r0 + rc, :], AT[0:rc, fj * N: fj * N + N])

        # ---------------- P3a: ctx, xx, LayerNorm ----------------
        with ExitStack() as p3:
            tmpb = p3.enter_context(tc.tile_pool(name="tmpb", bufs=1))
            tmpa = p3.enter_context(tc.tile_pool(name="tmpa", bufs=1))

            with ExitStack() as s1:
                ps_c = s1.enter_context(tc.tile_pool(name="ps_c", bufs=3, space="PSUM"))
                ctxT = tmpb.tile([H, N], f32, tag="ctxT")
                xxt = tmpb.tile([128, N], f32, tag="xxt")
                sqt = tmpa.tile([128, N], f32, tag="sqt")
                for o, w in _chunks512(N):
                    ps = ps_c.tile([H, 512], f32, tag="psc")
                    for fj, (jr0, jrc) in enumerate(CH):
                        nc.tensor.matmul(ps[0:H, 0:w],
                                         vnat[0:jrc, fj * H:(fj + 1) * H],
                                         AT[0:jrc, fj * N + o: fj * N + o + w],
                                         start=(fj == 0), stop=(fj == NCH - 1))
                    nc.scalar.activation(ctxT[0:H, o:o + w], ps[0:H, 0:w], AF.Copy)
                for o, w in _chunks512(N):
                    psx = ps_c.tile([128, 512], f32, tag="psx")
                    nc.tensor.matmul(psx[0:D, 0:w], sb["w_o"][:],
                                     ctxT[0:H, o:o + w], start=True, stop=True)
                    nc.scalar.activation(xxt[:, o:o + w], psx[0:D, 0:w], AF.Identity,
                                         bias=sb["bo"][:])
                    nc.vector.tensor_add(xxt[:, o:o + w], xxt[:, o:o + w],
                                         xsT[:, o:o + w])
                    nc.scalar.activation(sqt[:, o:o + w], xxt[:, o:o + w], AF.Square)

            with ExitStack() as s2:
                ps_l = s2.enter_context(tc.tile_pool(name="ps_l", bufs=1, space="PSUM"))
                psl = ps_l.tile([1, 2 * NPAD], f32, tag="psl")
                for o, w in _chunks512(N):
                    nc.tensor.matmul(psl[0:1, o:o + w], sb["ones128"][:],
                                     xxt[:, o:o + w], start=True, stop=True)
                    nc.tensor.matmul(psl[0:1, NPAD + o: NPAD + o + w],
                                     sb["ones128"][:], sqt[:, o:o + w],
                                     start=True, stop=True)
                # mu = colsum/128, msq = colsum(x^2)/128, isd = 1/sqrt(var+eps)
                lnv = tmpa.tile([1, 2 * NPAD], f32, tag="lnv")
                nc.vector.tensor_scalar(out=lnv[0:1, :], in0=psl[0:1, :],
                                        scalar1=1.0 / 128.0, scalar2=None,
                                        op0=OP.mult)
                lntmp = tmpa.tile([1, N], f32, tag="lntmp")
                nc.vector.tensor_mul(lntmp[0:1, :], lnv[0:1, 0:N], lnv[0:1, 0:N])
                nc.vector.tensor_sub(lnv[0:1, NPAD:NPAD + N],
                                     lnv[0:1, NPAD:NPAD + N], lntmp[0:1, :])
                nc.vector.tensor_scalar_add(lnv[0:1, NPAD:NPAD + N],
                                            lnv[0:1, NPAD:NPAD + N], 1e-6)
                # isd = 1/sqrt(var+eps) in one LUT (DVE reciprocal on a
                # 1-partition 2000-vector costs 8 cyc/elem = ~16us)
                nc.scalar.activation(lntmp[0:1, :], lnv[0:1, NPAD:NPAD + N],
                                     AF.Abs_reciprocal_sqrt)
                nc.vector.tensor_copy(lnv[0:1, NPAD:NPAD + N], lntmp[0:1, :])

                bc = tmpa.tile([128, 2 * NPAD], f32, tag="bc")
                nc.gpsimd.partition_broadcast(bc[0:128, 0:N], lnv[0:1, 0:N])
                nc.gpsimd.partition_broadcast(bc[0:128, NPAD:NPAD + N],
                                              lnv[0:1, NPAD:NPAD + N])
                nc.vector.tensor_sub(xxn[:, :], xxt[:, :], bc[:, 0:N])
                nc.vector.tensor_mul(xxn[:, :], xxn[:, :], bc[:, NPAD:NPAD + N])
                nc.vector.tensor_scalar(out=xxn[:, :], in0=xxn[:, :],
                                        scalar1=sb["lng"][:], scalar2=sb["lnb"][:],
                                        op0=OP.mult, op1=OP.add)

        # ---------------- P3b: gates and update ----------------
        with ExitStack() as p4:
            wrk = p4.enter_context(tc.tile_pool(name="wrk", bufs=1))

            # h12 = 0.5 * xxn @ [g1_W | g2_W]  (T-layout bf16 + natural bf16)
            h12T = wrk.tile([128, NPAD], bf16, tag="h12T")
            h12n = wrk.tile([128, NCH * D], bf16, tag="h12n")
            with ExitStack() as s3:
                ps_h = s3.enter_context(tc.tile_pool(name="ps_h", bufs=3, space="PSUM"))
                for o, w in _chunks512(N):
                    ps = ps_h.tile([128, 512], f32, tag="psh")
                    nc.tensor.matmul(ps[0:D, 0:w], sb["g12"][:],
                                     xxn[:, o:o + w], start=True, stop=True)
                    nc.scalar.activation(h12T[:, o:o + w], ps[0:D, 0:w],
                                         AF.Identity, scale=0.5)
                h12n3 = h12n[:].rearrange("p (f r) -> p f r", f=NCH)
                nc.sync.dma_start_transpose(out=h12n3, in_=h12T[0:D, 0:NPAD])

            # zr = sigmoid(attn @ h12 + h12 + gb12); z/r as separate base-0 tiles
            zT = wrk.tile([H, N], f32, tag="zT")
            rT = wrk.tile([H, N], f32, tag="rT")
            scr = wrk.tile([128, N], f32, tag="scr")
            with ExitStack() as s4:
                ps_m = s4.enter_context(tc.tile_pool(name="ps_m", bufs=3, space="PSUM"))
                for o, w in _chunks512(N):
                    ps = ps_m.tile([128, 512], f32, tag="psm")
                    for fj, (jr0, jrc) in enumerate(CH):
                        nc.tensor.matmul(ps[0:D, 0:w],
                                         h12n[0:jrc, fj * D:(fj + 1) * D],
                                         AT[0:jrc, fj * N + o: fj * N + o + w],
                                         start=(fj == 0), stop=(fj == NCH - 1))
                    nc.vector.tensor_add(scr[:, o:o + w], ps[0:D, 0:w],
                                         h12T[:, o:o + w])
                    nc.scalar.activation(zT[0:H, o:o + w], scr[0:H, o:o + w],
                                         AF.Sigmoid, bias=sb["gb12"][0:H, :])
                    nc.scalar.activation(rT[0:H, o:o + w], scr[64:128, o:o + w],
                                         AF.Sigmoid, bias=sb["gb12"][64:128, :])

            # h3 = 0.5 * (xxn @ up1 + (z*state) @ up2)
            zst = wrk.tile([H, N], f32, tag="zst")
            nc.vector.tensor_mul(zst[0:H, :], zT[0:H, :], stT0[0:H, :])
            h3T = wrk.tile([H, NPAD], bf16, tag="h3T")
            h3n = wrk.tile([128, NCH * H], bf16, tag="h3n")
            with ExitStack() as s5:
                ps_3 = s5.enter_context(tc.tile_pool(name="ps_3", bufs=3, space="PSUM"))
                for o, w in _chunks512(N):
                    ps = ps_3.tile([H, 512], f32, tag="ps3")
                    nc.tensor.matmul(ps[0:H, 0:w], sb["up1"][:],
                                     xxn[:, o:o + w], start=True, stop=False)
                    nc.tensor.matmul(ps[0:H, 0:w], sb["up2"][:],
                                     zst[0:H, o:o + w], start=False, stop=True)
                    nc.scalar.activation(h3T[0:H, o:o + w], ps[0:H, 0:w],
                                         AF.Identity, scale=0.5)
                h3n3 = h3n[:].rearrange("p (f r) -> p f r", f=NCH)
                nc.sync.dma_start_transpose(out=h3n3, in_=h3T[0:H, 0:NPAD])

            # hc = tanh(attn @ h3 + h3 + up_b);  h = hc + r*(state - hc)
            hcT = wrk.tile([H, N], f32, tag="hcT")
            hT = wrk.tile([H, N], f32, tag="hT")
            with ExitStack() as s6:
                ps_m3 = s6.enter_context(tc.tile_pool(name="ps_m3", bufs=3, space="PSUM"))
                for o, w in _chunks512(N):
                    ps = ps_m3.tile([H, 512], f32, tag="psm3")
                    for fj, (jr0, jrc) in enumerate(CH):
                        nc.tensor.matmul(ps[0:H, 0:w],
                                         h3n[0:jrc, fj * H:(fj + 1) * H],
                                         AT[0:jrc, fj * N + o: fj * N + o + w],
                                         start=(fj == 0), stop=(fj == NCH - 1))
                    nc.vector.tensor_add(scr[0:H, o:o + w], ps[0:H, 0:w],
                                         h3T[0:H, o:o + w])
                    nc.scalar.activation(hcT[0:H, o:o + w], scr[0:H, o:o + w],
                                         AF.Tanh, bias=sb["upb"][:])
            nc.vector.tensor_sub(hT[0:H, :], stT0[0:H, :], hcT[0:H, :])
            nc.vector.tensor_mul(hT[0:H, :], hT[0:H, :], rT[0:H, :])
            nc.vector.tensor_add(hT[0:H, :], hT[0:H, :], hcT[0:H, :])

            # h output: transpose to natural layout and DMA out
            with ExitStack() as s7:
                ps_t4 = s7.enter_context(tc.tile_pool(name="ps_t4", bufs=2, space="PSUM"))
                hn = p4.enter_context(tc.tile_pool(name="hn", bufs=2))
                for fj, (r0, rc) in enumerate(CH):
                    pst = ps_t4.tile([128, H], f32, tag="pst4")
                    nc.tensor.transpose(pst[0:rc, 0:H], hT[0:H, r0:r0 + rc],
                                        sb["ident"][0:H, 0:H])
                    hnt = hn.tile([128, H], f32, tag="hnt")
                    nc.scalar.activation(hnt[0:rc, :], pst[0:rc, 0:H], AF.Copy)
                    nc.sync.dma_start(h_out[r0:r0 + rc, :], hnt[0:rc, :])

    nc.compile()
    return nc


def _prep_shards(inputs):
    g = lambda k: np.ascontiguousarray(np.asarray(inputs[k]), dtype=np.float32)
    x, state = g("x"), g("state")
    B = x.shape[0]
    assert x.shape == (B, N, DIN) and state.shape == (B, N, H)
    assert int(np.asarray(inputs["top_k"])) == K

    col = lambda a: np.asarray(a, np.float32).reshape(-1, 1)
    mats = {
        "lin_W": g("lin_W"), "w_q": g("Wq"), "w_k": g("Wk"), "w_v": g("Wv"),
        "w_o": g("Wo"),
        "g12": np.concatenate([g("g1_W"), g("g2_W")], axis=1),
        "up1": g("up_W")[:D], "up2": g("up_W")[D:],
        "linb": col(inputs["lin_b"]), "bq8": col(np.asarray(inputs["bq"]) / 8.0),
        "bk": col(inputs["bk"]), "bv": col(inputs["bv"]), "bo": col(inputs["bo"]),
        "gb12": col(np.concatenate([np.asarray(inputs["g1_b"]),
                                    np.asarray(inputs["g2_b"])])),
        "upb": col(inputs["up_b"]),
        "lng": col(inputs["ln_g"]), "lnb": col(inputs["ln_b"]),
        "ident": np.eye(128, dtype=np.float32),
        "ones128": np.ones((128, 1), dtype=np.float32),
    }
    wpack = np.zeros((128, WPACK), dtype=np.float32)
    for name, (p, c0, w) in WLAYOUT.items():
        a = mats[name]
        assert a.shape == (p, w), (name, a.shape, (p, w))
        wpack[0:p, c0:c0 + w] = a
    common = {"wpack": wpack}
    in_maps = []
    for c in range(B):
        m = dict(common)
        m["xT"] = np.ascontiguousarray(x[c].T)
        m["stT"] = np.ascontiguousarray(state[c].T)
        in_maps.append(m)
    return in_maps, B


def kernel(**inputs):
    global _PROG
    from concourse.bass_utils import run_bass_kernel_spmd

    if _PROG is None:
        _PROG = _build_program()
    nc = _PROG

    in_maps, B = _prep_shards(inputs)
    assert B == 8, f"expected B=8, got {B}"
    res = run_bass_kernel_spmd(nc, in_maps, core_ids=list(range(B)))
    h = np.stack([res.results[c]["h_out"] for c in range(B)])
    A = np.stack([res.results[c]["A_out"] for c in range(B)])
    return h, A
